# revision 15
# baseline (speedup 1.0000x reference)
"""Trainium2 Bass kernel for nn_Cat_Linear_Encoder (pairwise MLP edge decoder).

probs[i,j] = sigmoid(W2 @ relu(W1 @ cat(z_i, z_j) + b1) + b2) * (1 - eye)

Host-side factorization (all O(N*H), exact):
    A[i,h] = |W2_h| * (z_i @ Wa.T + b1)[h]      (Wa = W1[:, :D])
    B[j,h] = |W2_h| * (z_j @ Wb.T)[h]           (Wb = W1[:, D:])
    s_h    = sign(W2_h)
    adj[i,j] = sum_h s_h * relu(A[i,h] + B[j,h]) + b2
using w*relu(x) == sign(w)*relu(|w|*x).

Device (per core, i-shard of 256 rows = 128 i-pairs):
    - R tile [128, 2048]: partitions = (pair-parity x 64 h), free = j.
      DVE tensor_scalar computes R = max(BdT + A_pair_column, 0) in one
      fused op (bf16 4x mode).
    - PE reduces h (partition axis) with a sliding 2-column sparse weight
      window: 64 accumulating matmuls build a [128 i, 512 j] PSUM tile
      holding adj for 128 rows.
    - ACT applies sigmoid PSUM->SBUF, DMA to DRAM.
Diagonal zeroing + shard concat happen on host.
"""

import numpy as np

N, D, H = 2048, 64, 64
NCORES = 8
SHARD = N // NCORES          # 256 i-rows per core
NPAIR = SHARD // 2           # 128 i-pairs per core
IBLK = SHARD // 128          # 2 psum row-blocks per core
JCH = 512                    # j-chunk = one PSUM bank of fp32
NJC = N // JCH               # 4

_CACHE = {}
_prepared_in_maps = None


def _build_bass(b2_val: float):
    import concourse.bacc as bacc
    import concourse.bass as bass
    import concourse.mybir as mybir
    from concourse.tile import TileContext

    bf16 = mybir.dt.bfloat16
    f32 = mybir.dt.float32

    nc = bacc.Bacc("TRN2", num_devices=NCORES)
    # one packed input DMA -> one semaphore; TS ISA struct allows only 1 wait
    PB = N * 2 + NPAIR * 4 + 256 * 2  # bytes/partition: bdt bf16 | apairs f32 | sbig bf16
    in_d = nc.dram_tensor("packed", [128, PB], mybir.dt.uint8, kind="ExternalInput")
    out_d = nc.dram_tensor("out", [SHARD, N], f32, kind="ExternalOutput")

    with TileContext(nc) as tc:
        with (
            tc.tile_pool(name="const", bufs=1) as cpool,
            tc.tile_pool(name="r", bufs=4) as rpool,
            tc.tile_pool(name="o", bufs=4) as opool,
            tc.tile_pool(name="psum", bufs=8, space=bass.MemorySpace.PSUM) as ppool,
        ):
            packed = cpool.tile([128, PB], mybir.dt.uint8, tag="packed")
            nc.gpsimd.dma_start(out=packed[:], in_=in_d[:])
            bdt = packed[:, 0 : N * 2].bitcast(bf16)
            apairs = packed[:, N * 2 : N * 2 + NPAIR * 4].bitcast(f32)
            sbig = packed[:, N * 2 + NPAIR * 4 : PB].bitcast(bf16)

            for ib in range(IBLK):
                ps = [
                    ppool.tile([128, JCH], f32, tag="ps", name=f"ps_{ib}_{jc}")
                    for jc in range(NJC)
                ]
                for l in range(64):
                    ip = ib * 64 + l
                    r = rpool.tile([128, N], bf16, tag="r")
                    nc.vector.tensor_scalar(
                        out=r[:],
                        in0=bdt,
                        scalar1=apairs[:, ip : ip + 1],
                        scalar2=0.0,
                        op0=mybir.AluOpType.add,
                        op1=mybir.AluOpType.max,
                    )
                    for jc in range(NJC):
                        nc.tensor.matmul(
                            ps[jc][:],
                            sbig[:, 128 - 2 * l : 256 - 2 * l],
                            r[:, jc * JCH : (jc + 1) * JCH],
                            start=(l == 0),
                            stop=(l == 63),
                        )
                for jc in range(NJC):
                    ot = opool.tile([128, JCH], f32, tag="ot")
                    nc.scalar.activation(
                        ot[:],
                        ps[jc][:],
                        mybir.ActivationFunctionType.Sigmoid,
                        bias=float(b2_val),
                    )
                    nc.sync.dma_start(
                        out=out_d[ib * 128 : (ib + 1) * 128, jc * JCH : (jc + 1) * JCH],
                        in_=ot[:],
                    )
    nc.compile()
    return nc


def _default_inputs():
    """Regenerate reference setup_inputs() deterministically (CPU jax)."""
    import jax

    cpu = jax.devices("cpu")[0]
    with jax.default_device(cpu):
        key = jax.random.key(0)
        k0, k1, k2 = jax.random.split(key, 3)
        z = np.asarray(jax.random.normal(k0, (N, D), dtype="float32"))
        W1 = np.asarray(
            jax.random.normal(k1, (H, 2 * D), dtype="float32")
            * np.float32(1.0 / np.sqrt(2 * D))
        )
        b1 = np.zeros((H,), dtype=np.float32)
        W2 = np.asarray(
            jax.random.normal(k2, (1, H), dtype="float32")
            * np.float32(1.0 / np.sqrt(H))
        )
        b2 = np.zeros((1,), dtype=np.float32)
    return z, W1, b1, W2, b2


def kernel(z=None, W1=None, b1=None, W2=None, b2=None, **_unused):
    from concourse import bass_utils

    if any(x is None for x in (z, W1, b1, W2, b2)):
        dz, dW1, db1, dW2, db2 = _default_inputs()
        z = dz if z is None else np.asarray(z)
        W1 = dW1 if W1 is None else np.asarray(W1)
        b1 = db1 if b1 is None else np.asarray(b1)
        W2 = dW2 if W2 is None else np.asarray(W2)
        b2 = db2 if b2 is None else np.asarray(b2)
    z = np.asarray(z, np.float32)
    W1 = np.asarray(W1, np.float32)
    b1 = np.asarray(b1, np.float32)
    W2 = np.asarray(W2, np.float32)
    b2 = np.asarray(b2, np.float32)

    Wa, Wb = W1[:, :D], W1[:, D:]
    w2 = W2[0]                                     # [H]
    s = np.where(w2 >= 0, 1.0, -1.0).astype(np.float32)
    aw = np.abs(w2)
    A = (z @ Wa.T + b1[None, :]) * aw[None, :]     # [N, H]
    B = (z @ Wb.T) * aw[None, :]                   # [N, H]

    import ml_dtypes

    bdt = np.ascontiguousarray(
        np.concatenate([B.T, B.T], axis=0).astype(ml_dtypes.bfloat16)
    )  # [128, N]

    sbig = np.zeros((128, 256), dtype=ml_dtypes.bfloat16)
    sbig[0:64, 128] = s.astype(ml_dtypes.bfloat16)
    sbig[64:128, 129] = s.astype(ml_dtypes.bfloat16)

    # per-core A-pair columns: core c owns i in [c*SHARD, (c+1)*SHARD)
    in_maps = []
    for c in range(NCORES):
        Ash = A[c * SHARD : (c + 1) * SHARD]       # [256, H]
        ap = np.empty((128, NPAIR), dtype=np.float32)
        ap[0:64, :] = Ash[0::2].T                  # even rows of shard
        ap[64:128, :] = Ash[1::2].T                # odd rows
        packed = np.concatenate(
            [
                bdt.view(np.uint8),                # [128, 2*N]
                np.ascontiguousarray(ap).view(np.uint8),
                sbig.view(np.uint8),
            ],
            axis=1,
        )
        in_maps.append({"packed": np.ascontiguousarray(packed)})

    global _prepared_in_maps
    _prepared_in_maps = in_maps

    key = float(b2[0])
    if key not in _CACHE:
        _CACHE[key] = _build_bass(key)
    nc = _CACHE[key]

    res = bass_utils.run_bass_kernel_spmd(nc, in_maps, core_ids=list(range(NCORES)))
    probs = np.concatenate([r["out"] for r in res.results], axis=0)
    probs[np.arange(N), np.arange(N)] = 0.0
    return probs.astype(np.float32)


if __name__ == "__main__":
    out = kernel()
    print(out.shape, out.dtype, out[:3, :3])
